# revision 5
# baseline (speedup 1.0000x reference)
"""Trainium2 Bass kernel for nn_Block_31954556682442 (spiking MoE-SSA block).

Sharding: pure data-parallel over batch B=8 -> one sample (4 LIF time steps)
per NeuronCore, zero collectives. Inside each core:
  - fp32 matmuls for all weight layers (k/v/q/router/proj/fc1/fc2)
  - bf16 exact-integer matmuls for the attention core (spikes are {0,1})
  - LIF scans in a 2^t-scaled form (bit-exact, powers of two): the membrane
    update becomes a plain tensor_add (runs on GPSIMD); spike/reset are
    tensor_scalar / scalar_tensor_tensor on DVE with threshold 2^t
  - PSUM evicts fused with BN scale+bias (+2^t*0.5 LIF prescale) on ScalarE
  - depthwise 3x3 conv as 9 shifted per-partition-scalar multiply-accumulates
Self-contained: hardcodes all shapes; no sibling imports.
"""
import numpy as np
import ml_dtypes

import concourse.bacc as bacc
import concourse.mybir as mybir
import concourse.tile as tile
from concourse.bass_utils import run_bass_kernel_spmd

F32 = mybir.dt.float32
BF16 = mybir.dt.bfloat16
AL = mybir.AluOpType
AF = mybir.ActivationFunctionType

T, B, C, N = 4, 8, 384, 256          # time steps, batch, channels, H*W
ED = 96                               # expert/k head dim
NE = 4                                # experts
NU = 5                                # stacked kq units (k + 4 experts)
HID, HH = 2048, 1024                  # MLP hidden, half
S = float(1.0 / np.sqrt(1.0 + 1e-5))  # BN eval scale
P = 128
USE_GPSIMD = True


def _body(nc, tc, d):
    from contextlib import ExitStack
    VE = nc.vector
    GE = nc.gpsimd if USE_GPSIMD else nc.vector

    def lif_add(R_tile, xp, t):
        """U_t = R_{t-1} + X_t in place over the evicted pre-act X (= xp)."""
        if t > 0:
            GE.tensor_add(xp, R_tile, xp)
        return xp

    def lif_reset(R_tile, U, t):
        """R_t = (U_t < 2^t) * U_t, skipped on the last step."""
        if t < T - 1:
            VE.scalar_tensor_tensor(
                out=R_tile, in0=U, scalar=float(2.0 ** t), in1=U,
                op0=AL.is_lt, op1=AL.mult)

    with ExitStack() as ctx:
        def pool(name, bufs, space="SBUF"):
            return ctx.enter_context(tc.tile_pool(name=name, bufs=bufs, space=space))

        wp = pool("wp", 1)
        mp = pool("mp", 1)
        ps_a = pool("ps_a", 2, "PSUM")
        ps_b = pool("ps_b", 3, "PSUM")
        ps_o = pool("ps_o", 3, "PSUM")
        xs_p = pool("xs_p", 12)
        xkq_p = pool("xkq_p", 10)
        xvt_p = pool("xvt_p", 5)
        xrt_p = pool("xrt_p", 8)
        ksp_p = pool("ksp_p", 4)
        qsp_p = pool("qsp_p", 16)
        vsp_p = pool("vsp_p", 8)
        wsp_p = pool("wsp_p", 8)
        xres_p = pool("xres_p", 5)
        rsp_p = pool("rsp_p", 3)
        y_p = pool("y_p", 8)
        atsb_p = pool("atsb_p", 3)
        ydn_p = pool("ydn_p", 4)
        xp_p = pool("xp_p", 6)
        xnew_p = pool("xnew_p", 6)
        xh_p = pool("xh_p", 6)
        x1_p = pool("x1_p", 3)
        x2_p = pool("x2_p", 3)
        acca_p = pool("acca_p", 3)
        mg_p = pool("mg_p", 4)
        xo_p = pool("xo_p", 6)
        osb_p = pool("osb_p", 4)

        # ---------------- weight loads ----------------
        def wload(name, shape, dt=F32, src=None):
            w = wp.tile(shape, dt, name=name, tag=name)
            nc.sync.dma_start(out=w, in_=d[name] if src is None else src)
            return w

        a_kq = wload('a_kq', [96, 20]); b_kq = wload('b_kq', [96, 20])
        rb = wload('r_b', [1, 4]); ones = wload('ones', [1, P])
        a_p = wload('a_p', [P, 12]); b_p = wload('b_p', [P, 12])
        a_h = wload('a_h', [P, 64]); b_h = wload('b_h', [P, 64])
        dwt = wload('dw_tap', [P, 288]); b_dw = wload('b_dw', [P, 32])
        a_o = wload('a_o', [P, 12]); b_o = wload('b_o', [P, 12])
        ident = wload('ident', [P, P], BF16)
        kq_w, v_w, r_w, pj_w, f1_w, f2_w = [], [], [], [], [], []
        for kt in range(3):
            kq_w.append(wload(f'kq_w{kt}', [P, 480], src=d['kq_wT'][kt*P:(kt+1)*P, :]))
            v_w.append(wload(f'v_w{kt}', [P, 384], src=d['v_wT'][kt*P:(kt+1)*P, :]))
            r_w.append(wload(f'r_w{kt}', [P, 4], src=d['r_wT'][kt*P:(kt+1)*P, :]))
            pj_w.append(wload(f'pj_w{kt}', [P, 384], src=d['proj_wT'][kt*P:(kt+1)*P, :]))
        # xs input tiles
        xs = [[None] * 3 for _ in range(T)]
        for t in range(T):
            for kt in range(3):
                x_ = xs_p.tile([P, N], F32, name=f"xs{t}_{kt}", tag="t")
                nc.sync.dma_start(out=x_, in_=d['xin'][t*C + kt*P: t*C + (kt+1)*P, :])
                xs[t][kt] = x_
        for kt in range(3):
            f1_w.append(wload(f'f1_w{kt}', [P, 2048], src=d['fc1_wT'][kt*P:(kt+1)*P, :]))
        for kt in range(8):
            f2_w.append(wload(f'f2_w{kt}', [P, 384], src=d['fc2_wT'][kt*P:(kt+1)*P, :]))

        # ---------------- phase A+B: k/q/v/router pre-acts + LIF ----------------
        m_kq = [mp.tile([96, N], F32, name=f"m_kq{u}", tag=f"m_kq{u}") for u in range(NU)]
        m_vt = [mp.tile([P, 384], F32, name=f"m_vt{i}", tag=f"m_vt{i}") for i in range(2)]
        m_rt = [mp.tile([P, 4], F32, name=f"m_rt{i}", tag=f"m_rt{i}") for i in range(2)]
        k_sp = [None] * T
        q_sp = [[None] * T for _ in range(NE)]
        v_sp = [[None] * 2 for _ in range(T)]
        w_sp = [[None] * 2 for _ in range(T)]

        for t in range(T):
            thr = float(2.0 ** t)
            for u in range(NU):
                pt = ps_a.tile([96, N], F32, name=f"pkq{t}_{u}", tag="pa")
                for kt in range(3):
                    nc.tensor.matmul(pt, kq_w[kt][:, 96*u:96*(u+1)], xs[t][kt],
                                     start=(kt == 0), stop=(kt == 2))
                xp = xkq_p.tile([96, N], F32, name=f"xkq{t}_{u}", tag="t")
                c = u * 4 + t
                nc.scalar.activation(xp, pt, AF.Identity,
                                     bias=b_kq[:, c:c+1], scale=a_kq[:, c:c+1])
                U = lif_add(m_kq[u], xp, t)
                spool = ksp_p if u == 0 else qsp_p
                sp = spool.tile([96, N], BF16, name=f"sp{t}_{u}", tag="t")
                VE.tensor_single_scalar(sp, U, thr, AL.is_ge)
                lif_reset(m_kq[u], U, t)
                if u == 0:
                    k_sp[t] = sp
                else:
                    q_sp[u-1][t] = sp
            for mt in range(2):
                pt = ps_a.tile([P, 384], F32, name=f"pvt{t}_{mt}", tag="pa")
                for kt in range(3):
                    nc.tensor.matmul(pt, xs[t][kt][:, mt*P:(mt+1)*P], v_w[kt],
                                     start=(kt == 0), stop=(kt == 2))
                xp = xvt_p.tile([P, 384], F32, name=f"xvt{t}_{mt}", tag="t")
                nc.scalar.activation(xp, pt, AF.Copy, bias=0.0, scale=0.5 * thr)
                U = lif_add(m_vt[mt], xp, t)
                vs = vsp_p.tile([P, 384], BF16, name=f"vsp{t}_{mt}", tag="t")
                VE.tensor_single_scalar(vs, U, thr, AL.is_ge)
                lif_reset(m_vt[mt], U, t)
                v_sp[t][mt] = vs
            for mt in range(2):
                pt = ps_a.tile([P, 4], F32, name=f"prt{t}_{mt}", tag="pa")
                for kt in range(3):
                    nc.tensor.matmul(pt, xs[t][kt][:, mt*P:(mt+1)*P], r_w[kt],
                                     start=(kt == 0), stop=False)
                nc.tensor.matmul(pt, ones, rb, start=False, stop=True)
                xp = xrt_p.tile([P, 4], F32, name=f"xrt{t}_{mt}", tag="t")
                nc.scalar.activation(xp, pt, AF.Copy, bias=0.0, scale=thr)
                U = lif_add(m_rt[mt], xp, t)
                ws = wsp_p.tile([P, 4], F32, name=f"wsp{t}_{mt}", tag="t")
                VE.tensor_single_scalar(ws, U, thr, AL.is_ge)
                lif_reset(m_rt[mt], U, t)
                w_sp[t][mt] = ws

        # ---------------- phase C: experts (bf16 exact integer core) ----------------
        m_res = [mp.tile([P, 384], F32, name=f"m_res{i}", tag=f"m_res{i}") for i in range(2)]
        y = [[None] * 2 for _ in range(T)]
        for e in range(NE):
            for t in range(T):
                thr = float(2.0 ** t)
                at_sb = []
                for mt in range(2):
                    pt = ps_b.tile([P, N], F32, name=f"pat{e}{t}{mt}", tag="pb")
                    nc.tensor.matmul(pt, k_sp[t][:, mt*P:(mt+1)*P], q_sp[e][t],
                                     start=True, stop=True)
                    ats = atsb_p.tile([P, N], BF16, name=f"at{e}{t}{mt}", tag="t")
                    nc.scalar.activation(ats, pt, AF.Copy)
                    at_sb.append(ats)
                for mt in range(2):
                    pr = ps_b.tile([P, 384], F32, name=f"pres{e}{t}{mt}", tag="pb")
                    for mk in range(2):
                        nc.tensor.matmul(pr, at_sb[mk][:, mt*P:(mt+1)*P], v_sp[t][mk],
                                         start=(mk == 0), stop=(mk == 1))
                    xr = xres_p.tile([P, 384], F32, name=f"xres{e}{t}{mt}", tag="t")
                    nc.scalar.activation(xr, pr, AF.Copy, bias=0.0, scale=0.5 * thr)
                    U = lif_add(m_res[mt], xr, t)
                    rs = rsp_p.tile([P, 384], BF16, name=f"rsp{e}{t}{mt}", tag="t")
                    VE.tensor_single_scalar(rs, U, thr, AL.is_ge)
                    lif_reset(m_res[mt], U, t)
                    if e == 0:
                        yt = y_p.tile([P, 384], BF16, name=f"y{t}_{mt}", tag="t")
                        VE.scalar_tensor_tensor(
                            out=yt, in0=rs, scalar=w_sp[t][mt][:, 0:1], in1=rs,
                            op0=AL.mult, op1=AL.bypass)
                        y[t][mt] = yt
                    else:
                        VE.scalar_tensor_tensor(
                            out=y[t][mt], in0=rs, scalar=w_sp[t][mt][:, e:e+1],
                            in1=y[t][mt], op0=AL.mult, op1=AL.add)

        # ---------------- phase D+E per t ----------------
        m_p = [mp.tile([P, N], F32, name=f"m_p{i}", tag=f"m_p{i}") for i in range(3)]
        m_h = [mp.tile([P, N], F32, name=f"m_h{i}", tag=f"m_h{i}") for i in range(16)]
        m_dw = [mp.tile([P, N], F32, name=f"m_dw{i}", tag=f"m_dw{i}") for i in range(8)]
        m_o = [mp.tile([P, N], F32, name=f"m_o{i}", tag=f"m_o{i}") for i in range(3)]

        for t in range(T):
            thr = float(2.0 ** t)
            # transpose y (n-major bf16) -> y_dn (d-major fp32)
            ydn = [ydn_p.tile([P, N], F32, name=f"ydn{t}_{dt}", tag="t") for dt in range(3)]
            for mt in range(2):
                for dt in range(3):
                    ptr = ps_b.tile([P, P], BF16, name=f"ptr{t}{mt}{dt}", tag="pb")
                    nc.tensor.transpose(ptr, y[t][mt][:, dt*P:(dt+1)*P], ident)
                    nc.scalar.activation(ydn[dt][:, mt*P:(mt+1)*P], ptr, AF.Copy)
            # proj + LIF + residual
            x_new = [None] * 3
            for mt in range(3):
                pp = ps_b.tile([P, N], F32, name=f"pp{t}_{mt}", tag="pb")
                for dt in range(3):
                    nc.tensor.matmul(pp, pj_w[dt][:, mt*P:(mt+1)*P], ydn[dt],
                                     start=(dt == 0), stop=(dt == 2))
                xp = xp_p.tile([P, N], F32, name=f"xp{t}_{mt}", tag="t")
                c = mt * 4 + t
                nc.scalar.activation(xp, pp, AF.Identity,
                                     bias=b_p[:, c:c+1], scale=a_p[:, c:c+1])
                U = lif_add(m_p[mt], xp, t)
                xn = xnew_p.tile([P, N], F32, name=f"xn{t}_{mt}", tag="t")
                VE.scalar_tensor_tensor(out=xn, in0=U, scalar=thr, in1=xs[t][mt],
                                        op0=AL.is_ge, op1=AL.add)
                lif_reset(m_p[mt], U, t)
                x_new[mt] = xn
            # MLP: fc1 -> LIF -> dwconv -> LIF*gate -> fc2 (accumulated per chunk)
            po = [ps_o.tile([P, N], F32, name=f"po{t}_{mt}", tag="po") for mt in range(3)]
            for ch in range(8):
                sp01 = [None, None]
                for half in range(2):
                    mt_h = ch + 8 * half
                    ph = ps_b.tile([P, N], F32, name=f"ph{t}{ch}{half}", tag="pb")
                    for dt in range(3):
                        nc.tensor.matmul(ph, f1_w[dt][:, mt_h*P:(mt_h+1)*P], x_new[dt],
                                         start=(dt == 0), stop=(dt == 2))
                    xh = xh_p.tile([P, N], F32, name=f"xh{t}{ch}{half}", tag="t")
                    c = mt_h * 4 + t
                    nc.scalar.activation(xh, ph, AF.Identity,
                                         bias=b_h[:, c:c+1], scale=a_h[:, c:c+1])
                    U = lif_add(m_h[mt_h], xh, t)
                    spool = x1_p if half == 0 else x2_p
                    sp = spool.tile([P, N], BF16, name=f"hsp{t}{ch}{half}", tag="t")
                    VE.tensor_single_scalar(sp, U, thr, AL.is_ge)
                    lif_reset(m_h[mt_h], U, t)
                    sp01[half] = sp
                x1s, x2s = sp01
                # depthwise 3x3: single DVE accumulate chain (taps pre-scaled 2^t)
                acc = acca_p.tile([P, N], F32, name=f"acc{t}_{ch}", tag="t")
                c0 = (ch * 9 + 4) * 4 + t
                cb = ch * 4 + t
                VE.tensor_scalar(acc, x1s, dwt[:, c0:c0+1], b_dw[:, cb:cb+1],
                                 AL.mult, AL.add)
                x1_3 = x1s.rearrange("p (h w) -> p h w", h=16)
                a3 = acc.rearrange("p (h w) -> p h w", h=16)
                for dy in range(3):
                    for dx in range(3):
                        if (dy, dx) == (1, 1):
                            continue
                        oy0, oy1 = max(0, 1-dy), min(16, 17-dy)
                        ox0, ox1 = max(0, 1-dx), min(16, 17-dx)
                        src = x1_3[:, oy0+dy-1:oy1+dy-1, ox0+dx-1:ox1+dx-1]
                        dsts = a3[:, oy0:oy1, ox0:ox1]
                        ct = (ch * 9 + 3 * dy + dx) * 4 + t
                        VE.scalar_tensor_tensor(
                            out=dsts, in0=src, scalar=dwt[:, ct:ct+1],
                            in1=dsts, op0=AL.mult, op1=AL.add)
                U = lif_add(m_dw[ch], acc, t)
                mg = mg_p.tile([P, N], F32, name=f"mg{t}_{ch}", tag="t")
                VE.scalar_tensor_tensor(out=mg, in0=U, scalar=thr, in1=x2s,
                                        op0=AL.is_ge, op1=AL.mult)
                lif_reset(m_dw[ch], U, t)
                for mt in range(3):
                    nc.tensor.matmul(po[mt], f2_w[ch][:, mt*P:(mt+1)*P], mg,
                                     start=(ch == 0), stop=(ch == 7),
                                     skip_group_check=True)
            # fc2 evict + final LIF + residual + store
            for mt in range(3):
                xo = xo_p.tile([P, N], F32, name=f"xo{t}_{mt}", tag="t")
                c = mt * 4 + t
                nc.scalar.activation(xo, po[mt], AF.Identity,
                                     bias=b_o[:, c:c+1], scale=a_o[:, c:c+1])
                U = lif_add(m_o[mt], xo, t)
                ob = osb_p.tile([P, N], F32, name=f"ob{t}_{mt}", tag="t")
                VE.scalar_tensor_tensor(out=ob, in0=U, scalar=thr, in1=x_new[mt],
                                        op0=AL.is_ge, op1=AL.add)
                lif_reset(m_o[mt], U, t)
                nc.sync.dma_start(out=d['out'][t*C + mt*P: t*C + (mt+1)*P, :], in_=ob)


def _build():
    nc = bacc.Bacc()
    with tile.TileContext(nc) as tc:
        with tc.tile_pool(name="dram", bufs=1, space="DRAM") as dram:
            def din(name, shape, dt=F32):
                return dram.tile(shape, dt, kind="ExternalInput", name=name,
                                 uniquify=False)
            d = {
                'xin': din('xin', [T * C, N]),
                'out': dram.tile([T * C, N], F32, kind="ExternalOutput",
                                 name='out', uniquify=False),
                'kq_wT': din('kq_wT', [384, 480]),
                'a_kq': din('a_kq', [96, 20]),
                'b_kq': din('b_kq', [96, 20]),
                'v_wT': din('v_wT', [384, 384]),
                'r_wT': din('r_wT', [384, 4]),
                'r_b': din('r_b', [1, 4]),
                'ones': din('ones', [1, 128]),
                'proj_wT': din('proj_wT', [384, 384]),
                'a_p': din('a_p', [128, 12]),
                'b_p': din('b_p', [128, 12]),
                'fc1_wT': din('fc1_wT', [384, 2048]),
                'a_h': din('a_h', [128, 64]),
                'b_h': din('b_h', [128, 64]),
                'dw_tap': din('dw_tap', [128, 288]),
                'b_dw': din('b_dw', [128, 32]),
                'fc2_wT': din('fc2_wT', [1024, 384]),
                'a_o': din('a_o', [128, 12]),
                'b_o': din('b_o', [128, 12]),
                'ident': din('ident', [128, 128], BF16),
            }
            _body(nc, tc, d)
    nc.finalize()
    return nc


_NC_CACHE = {}


def _get_nc():
    if 'nc' not in _NC_CACHE:
        _NC_CACHE['nc'] = _build()
    return _NC_CACHE['nc']


def _tcols(a):
    """(rows, k) per-unit scalars -> (rows, 4k) with column u*4+t = 2^t * a[:,u]."""
    rows, k = a.shape
    out = np.empty((rows, k * 4), np.float32)
    for u in range(k):
        for t in range(4):
            out[:, u * 4 + t] = a[:, u] * (2.0 ** t)
    return out


def _prep_common(inputs):
    inp = {k: np.asarray(v, np.float32) for k, v in inputs.items()}
    k_wT = inp['k_w'].T
    exp_wT = np.concatenate([inp['exp_w'][e].T for e in range(NE)], axis=1)
    kq_wT = np.concatenate([k_wT, exp_wT], axis=1)
    a_kq = np.zeros((96, 5), np.float32)
    b_kq = np.zeros((96, 5), np.float32)
    a_kq[:, 0] = 0.5
    for e in range(NE):
        a_kq[:, 1 + e] = 0.5 * inp['exp_g'][e] * S
        b_kq[:, 1 + e] = 0.5 * inp['exp_b'][e]
    taps = inp['dw_w'][:, 0] * (0.5 * inp['dw_g'] * S)[:, None, None]
    com = {
        'kq_wT': kq_wT,
        'a_kq': _tcols(a_kq), 'b_kq': _tcols(b_kq),
        'v_wT': inp['v_w'].T,
        'r_wT': inp['router_w'].T * (inp['router_g'] * S * 0.5)[None, :],
        'r_b': (0.5 * (inp['router_b'] * inp['router_g'] * S
                       + inp['router_be'])).reshape(1, 4),
        'ones': np.ones((1, 128), np.float32),
        'proj_wT': inp['proj_w'].T,
        'a_p': _tcols((0.5 * inp['proj_g'] * S).reshape(3, 128).T),
        'b_p': _tcols((0.5 * (inp['proj_b'] * inp['proj_g'] * S
                              + inp['proj_be'])).reshape(3, 128).T),
        'fc1_wT': inp['fc1_w'].T,
        'a_h': _tcols((0.5 * inp['fc1_g'] * S).reshape(16, 128).T),
        'b_h': _tcols((0.5 * (inp['fc1_b'] * inp['fc1_g'] * S
                              + inp['fc1_be'])).reshape(16, 128).T),
        'dw_tap': _tcols(taps.reshape(8, 128, 9).transpose(1, 0, 2).reshape(128, 72)),
        'b_dw': _tcols((0.5 * (inp['dw_b'] * inp['dw_g'] * S
                               + inp['dw_be'])).reshape(8, 128).T),
        'fc2_wT': inp['fc2_w'].T,
        'a_o': _tcols((0.5 * inp['fc2_g'] * S).reshape(3, 128).T),
        'b_o': _tcols((0.5 * (inp['fc2_b'] * inp['fc2_g'] * S
                              + inp['fc2_be'])).reshape(3, 128).T),
        'ident': np.eye(128, dtype=ml_dtypes.bfloat16),
    }
    return {k: np.ascontiguousarray(v) for k, v in com.items()}


def run(inputs, trace=False):
    com = _prep_common(inputs)
    x = np.asarray(inputs['x'], np.float32).reshape(T, B, C, N)
    in_maps = []
    for b in range(B):
        m = dict(com)
        m['xin'] = np.ascontiguousarray(x[:, b].reshape(T * C, N))
        in_maps.append(m)
    res = run_bass_kernel_spmd(_get_nc(), in_maps, list(range(B)), trace=trace)
    out = np.empty((T, B, C, N), np.float32)
    for b in range(B):
        out[:, b] = res.results[b]['out'].reshape(T, C, N)
    return out.reshape(T * B, C, 16, 16), res.exec_time_ns


def kernel(**inputs):
    out, _ = run(inputs)
    return out


# revision 6
# speedup vs baseline: 4381.8219x; 4381.8219x over previous
"""Trainium2 Bass kernel for nn_Block_31954556682442 (spiking MoE-SSA block).

Sharding: pure data-parallel over batch B=8 -> one sample (4 LIF time steps)
per NeuronCore, zero collectives. Inside each core:
  - fp32 matmuls for all weight layers (k/v/q/router/proj/fc1/fc2)
  - bf16 exact-integer matmuls for the attention core (spikes are {0,1})
  - LIF scans in a 2^t-scaled form (bit-exact, powers of two): the membrane
    update becomes a plain tensor_add (runs on GPSIMD); spike/reset are
    tensor_scalar / scalar_tensor_tensor on DVE with threshold 2^t
  - PSUM evicts fused with BN scale+bias (+2^t*0.5 LIF prescale) on ScalarE
  - depthwise 3x3 conv as 9 shifted per-partition-scalar multiply-accumulates
Self-contained: hardcodes all shapes; no sibling imports.
"""
import numpy as np
import ml_dtypes

import concourse.bacc as bacc
import concourse.mybir as mybir
import concourse.tile as tile
from concourse.bass_utils import run_bass_kernel_spmd

F32 = mybir.dt.float32
BF16 = mybir.dt.bfloat16
AL = mybir.AluOpType
AF = mybir.ActivationFunctionType

T, B, C, N = 4, 8, 384, 256          # time steps, batch, channels, H*W
ED = 96                               # expert/k head dim
NE = 4                                # experts
NU = 5                                # stacked kq units (k + 4 experts)
HID, HH = 2048, 1024                  # MLP hidden, half
S = float(1.0 / np.sqrt(1.0 + 1e-5))  # BN eval scale
P = 128
USE_GPSIMD = True


def _body(nc, tc, d):
    from contextlib import ExitStack
    VE = nc.vector
    GE = nc.gpsimd if USE_GPSIMD else nc.vector

    def lif_add(R_tile, xp, t):
        """U_t = R_{t-1} + X_t in place over the evicted pre-act X (= xp)."""
        if t > 0:
            GE.tensor_add(xp, R_tile, xp)
        return xp

    def lif_reset(R_tile, U, t):
        """R_t = (U_t < 2^t) * U_t, skipped on the last step."""
        if t < T - 1:
            VE.scalar_tensor_tensor(
                out=R_tile, in0=U, scalar=float(2.0 ** t), in1=U,
                op0=AL.is_lt, op1=AL.mult)

    with ExitStack() as ctx:
        def pool(name, bufs, space="SBUF"):
            return ctx.enter_context(tc.tile_pool(name=name, bufs=bufs, space=space))

        wp = pool("wp", 1)
        mp = pool("mp", 1)
        ps_a = pool("ps_a", 2, "PSUM")
        ps_b = pool("ps_b", 3, "PSUM")
        ps_o = pool("ps_o", 3, "PSUM")
        xs_p = pool("xs_p", 12)
        xkq_p = pool("xkq_p", 10)
        xvt_p = pool("xvt_p", 5)
        xrt_p = pool("xrt_p", 8)
        ksp_p = pool("ksp_p", 4)
        qsp_p = pool("qsp_p", 16)
        vsp_p = pool("vsp_p", 8)
        wsp_p = pool("wsp_p", 8)
        xres_p = pool("xres_p", 5)
        rsp_p = pool("rsp_p", 3)
        y_p = pool("y_p", 8)
        atsb_p = pool("atsb_p", 3)
        ydn_p = pool("ydn_p", 4)
        xp_p = pool("xp_p", 6)
        xnew_p = pool("xnew_p", 6)
        xh_p = pool("xh_p", 6)
        x1_p = pool("x1_p", 3)
        x2_p = pool("x2_p", 3)
        acca_p = pool("acca_p", 3)
        mg_p = pool("mg_p", 4)
        xo_p = pool("xo_p", 6)
        osb_p = pool("osb_p", 4)

        # ---------------- weight loads ----------------
        def wload(name, shape, dt=F32, src=None):
            w = wp.tile(shape, dt, name=name, tag=name)
            nc.sync.dma_start(out=w, in_=d[name] if src is None else src)
            return w

        a_kq = wload('a_kq', [96, 20]); b_kq = wload('b_kq', [96, 20])
        rb = wload('r_b', [1, 4]); ones = wload('ones', [1, P])
        a_p = wload('a_p', [P, 12]); b_p = wload('b_p', [P, 12])
        a_h = wload('a_h', [P, 64]); b_h = wload('b_h', [P, 64])
        dwt = wload('dw_tap', [P, 288]); b_dw = wload('b_dw', [P, 32])
        a_o = wload('a_o', [P, 12]); b_o = wload('b_o', [P, 12])
        ident = wload('ident', [P, P], BF16)
        kq_w, v_w, r_w, pj_w, f1_w, f2_w = [], [], [], [], [], []
        for kt in range(3):
            kq_w.append(wload(f'kq_w{kt}', [P, 480], src=d['kq_wT'][kt*P:(kt+1)*P, :]))
            v_w.append(wload(f'v_w{kt}', [P, 384], src=d['v_wT'][kt*P:(kt+1)*P, :]))
            r_w.append(wload(f'r_w{kt}', [P, 4], src=d['r_wT'][kt*P:(kt+1)*P, :]))
            pj_w.append(wload(f'pj_w{kt}', [P, 384], src=d['proj_wT'][kt*P:(kt+1)*P, :]))
        # xs input tiles
        xs = [[None] * 3 for _ in range(T)]
        for t in range(T):
            for kt in range(3):
                x_ = xs_p.tile([P, N], F32, name=f"xs{t}_{kt}", tag="t")
                nc.sync.dma_start(out=x_, in_=d['xin'][t*C + kt*P: t*C + (kt+1)*P, :])
                xs[t][kt] = x_
        for kt in range(3):
            f1_w.append(wload(f'f1_w{kt}', [P, 2048], src=d['fc1_wT'][kt*P:(kt+1)*P, :]))
        for kt in range(8):
            f2_w.append(wload(f'f2_w{kt}', [P, 384], src=d['fc2_wT'][kt*P:(kt+1)*P, :]))

        # ---------------- phase A+B: k/q/v/router pre-acts + LIF ----------------
        m_kq = [mp.tile([96, N], F32, name=f"m_kq{u}", tag=f"m_kq{u}") for u in range(NU)]
        m_vt = [mp.tile([P, 384], F32, name=f"m_vt{i}", tag=f"m_vt{i}") for i in range(2)]
        m_rt = [mp.tile([P, 4], F32, name=f"m_rt{i}", tag=f"m_rt{i}") for i in range(2)]
        k_sp = [None] * T
        q_sp = [[None] * T for _ in range(NE)]
        v_sp = [[None] * 2 for _ in range(T)]
        w_sp = [[None] * 2 for _ in range(T)]

        for t in range(T):
            thr = float(2.0 ** t)
            for u in range(NU):
                pt = ps_a.tile([96, N], F32, name=f"pkq{t}_{u}", tag="pa")
                for kt in range(3):
                    nc.tensor.matmul(pt, kq_w[kt][:, 96*u:96*(u+1)], xs[t][kt],
                                     start=(kt == 0), stop=(kt == 2))
                xp = xkq_p.tile([96, N], F32, name=f"xkq{t}_{u}", tag="t")
                c = u * 4 + t
                nc.scalar.activation(xp, pt, AF.Identity,
                                     bias=b_kq[:, c:c+1], scale=a_kq[:, c:c+1])
                U = lif_add(m_kq[u], xp, t)
                spool = ksp_p if u == 0 else qsp_p
                sp = spool.tile([96, N], BF16, name=f"sp{t}_{u}", tag="t")
                VE.tensor_single_scalar(sp, U, thr, AL.is_ge)
                lif_reset(m_kq[u], U, t)
                if u == 0:
                    k_sp[t] = sp
                else:
                    q_sp[u-1][t] = sp
            for mt in range(2):
                pt = ps_a.tile([P, 384], F32, name=f"pvt{t}_{mt}", tag="pa")
                for kt in range(3):
                    nc.tensor.matmul(pt, xs[t][kt][:, mt*P:(mt+1)*P], v_w[kt],
                                     start=(kt == 0), stop=(kt == 2))
                xp = xvt_p.tile([P, 384], F32, name=f"xvt{t}_{mt}", tag="t")
                nc.scalar.activation(xp, pt, AF.Copy, bias=0.0, scale=0.5 * thr)
                U = lif_add(m_vt[mt], xp, t)
                vs = vsp_p.tile([P, 384], BF16, name=f"vsp{t}_{mt}", tag="t")
                VE.tensor_single_scalar(vs, U, thr, AL.is_ge)
                lif_reset(m_vt[mt], U, t)
                v_sp[t][mt] = vs
            for mt in range(2):
                pt = ps_a.tile([P, 4], F32, name=f"prt{t}_{mt}", tag="pa")
                for kt in range(3):
                    nc.tensor.matmul(pt, xs[t][kt][:, mt*P:(mt+1)*P], r_w[kt],
                                     start=(kt == 0), stop=False)
                nc.tensor.matmul(pt, ones, rb, start=False, stop=True)
                xp = xrt_p.tile([P, 4], F32, name=f"xrt{t}_{mt}", tag="t")
                nc.scalar.activation(xp, pt, AF.Copy, bias=0.0, scale=thr)
                U = lif_add(m_rt[mt], xp, t)
                ws = wsp_p.tile([P, 4], F32, name=f"wsp{t}_{mt}", tag="t")
                VE.tensor_single_scalar(ws, U, thr, AL.is_ge)
                lif_reset(m_rt[mt], U, t)
                w_sp[t][mt] = ws

        # ---------------- phase C: experts (bf16 exact integer core) ----------------
        m_res = [mp.tile([P, 384], F32, name=f"m_res{i}", tag=f"m_res{i}") for i in range(2)]
        y = [[None] * 2 for _ in range(T)]
        for e in range(NE):
            for t in range(T):
                thr = float(2.0 ** t)
                at_sb = []
                for mt in range(2):
                    pt = ps_b.tile([P, N], F32, name=f"pat{e}{t}{mt}", tag="pb")
                    nc.tensor.matmul(pt, k_sp[t][:, mt*P:(mt+1)*P], q_sp[e][t],
                                     start=True, stop=True)
                    ats = atsb_p.tile([P, N], BF16, name=f"at{e}{t}{mt}", tag="t")
                    nc.scalar.activation(ats, pt, AF.Copy)
                    at_sb.append(ats)
                for mt in range(2):
                    pr = ps_b.tile([P, 384], F32, name=f"pres{e}{t}{mt}", tag="pb")
                    for mk in range(2):
                        nc.tensor.matmul(pr, at_sb[mk][:, mt*P:(mt+1)*P], v_sp[t][mk],
                                         start=(mk == 0), stop=(mk == 1))
                    xr = xres_p.tile([P, 384], F32, name=f"xres{e}{t}{mt}", tag="t")
                    nc.scalar.activation(xr, pr, AF.Copy, bias=0.0, scale=0.5 * thr)
                    U = lif_add(m_res[mt], xr, t)
                    rs = rsp_p.tile([P, 384], BF16, name=f"rsp{e}{t}{mt}", tag="t")
                    VE.tensor_single_scalar(rs, U, thr, AL.is_ge)
                    lif_reset(m_res[mt], U, t)
                    if e == 0:
                        yt = y_p.tile([P, 384], BF16, name=f"y{t}_{mt}", tag="t")
                        VE.scalar_tensor_tensor(
                            out=yt, in0=rs, scalar=w_sp[t][mt][:, 0:1], in1=rs,
                            op0=AL.mult, op1=AL.bypass)
                        y[t][mt] = yt
                    else:
                        VE.scalar_tensor_tensor(
                            out=y[t][mt], in0=rs, scalar=w_sp[t][mt][:, e:e+1],
                            in1=y[t][mt], op0=AL.mult, op1=AL.add)

        # ---------------- phase D+E per t ----------------
        m_p = [mp.tile([P, N], F32, name=f"m_p{i}", tag=f"m_p{i}") for i in range(3)]
        m_h = [mp.tile([P, N], F32, name=f"m_h{i}", tag=f"m_h{i}") for i in range(16)]
        m_dw = [mp.tile([P, N], F32, name=f"m_dw{i}", tag=f"m_dw{i}") for i in range(8)]
        m_o = [mp.tile([P, N], F32, name=f"m_o{i}", tag=f"m_o{i}") for i in range(3)]

        for t in range(T):
            thr = float(2.0 ** t)
            # transpose y (n-major bf16) -> y_dn (d-major fp32)
            ydn = [ydn_p.tile([P, N], F32, name=f"ydn{t}_{dt}", tag="t") for dt in range(3)]
            for mt in range(2):
                for dt in range(3):
                    ptr = ps_b.tile([P, P], BF16, name=f"ptr{t}{mt}{dt}", tag="pb")
                    nc.tensor.transpose(ptr, y[t][mt][:, dt*P:(dt+1)*P], ident)
                    nc.scalar.activation(ydn[dt][:, mt*P:(mt+1)*P], ptr, AF.Copy)
            # proj + LIF + residual
            x_new = [None] * 3
            for mt in range(3):
                pp = ps_b.tile([P, N], F32, name=f"pp{t}_{mt}", tag="pb")
                for dt in range(3):
                    nc.tensor.matmul(pp, pj_w[dt][:, mt*P:(mt+1)*P], ydn[dt],
                                     start=(dt == 0), stop=(dt == 2))
                xp = xp_p.tile([P, N], F32, name=f"xp{t}_{mt}", tag="t")
                c = mt * 4 + t
                nc.scalar.activation(xp, pp, AF.Identity,
                                     bias=b_p[:, c:c+1], scale=a_p[:, c:c+1])
                U = lif_add(m_p[mt], xp, t)
                xn = xnew_p.tile([P, N], F32, name=f"xn{t}_{mt}", tag="t")
                VE.scalar_tensor_tensor(out=xn, in0=U, scalar=thr, in1=xs[t][mt],
                                        op0=AL.is_ge, op1=AL.add)
                lif_reset(m_p[mt], U, t)
                x_new[mt] = xn
            # MLP: fc1 -> LIF -> dwconv -> LIF*gate -> fc2 (accumulated per chunk)
            po = [ps_o.tile([P, N], F32, name=f"po{t}_{mt}", tag="po") for mt in range(3)]
            for ch in range(8):
                sp01 = [None, None]
                for half in range(2):
                    mt_h = ch + 8 * half
                    ph = ps_b.tile([P, N], F32, name=f"ph{t}{ch}{half}", tag="pb")
                    for dt in range(3):
                        nc.tensor.matmul(ph, f1_w[dt][:, mt_h*P:(mt_h+1)*P], x_new[dt],
                                         start=(dt == 0), stop=(dt == 2))
                    xh = xh_p.tile([P, N], F32, name=f"xh{t}{ch}{half}", tag="t")
                    c = mt_h * 4 + t
                    nc.scalar.activation(xh, ph, AF.Identity,
                                         bias=b_h[:, c:c+1], scale=a_h[:, c:c+1])
                    U = lif_add(m_h[mt_h], xh, t)
                    spool = x1_p if half == 0 else x2_p
                    sp = spool.tile([P, N], BF16, name=f"hsp{t}{ch}{half}", tag="t")
                    VE.tensor_single_scalar(sp, U, thr, AL.is_ge)
                    lif_reset(m_h[mt_h], U, t)
                    sp01[half] = sp
                x1s, x2s = sp01
                # depthwise 3x3: single DVE accumulate chain (taps pre-scaled 2^t)
                acc = acca_p.tile([P, N], F32, name=f"acc{t}_{ch}", tag="t")
                c0 = (ch * 9 + 4) * 4 + t
                cb = ch * 4 + t
                VE.tensor_scalar(acc, x1s, dwt[:, c0:c0+1], b_dw[:, cb:cb+1],
                                 AL.mult, AL.add)
                x1_3 = x1s.rearrange("p (h w) -> p h w", h=16)
                a3 = acc.rearrange("p (h w) -> p h w", h=16)
                for dy in range(3):
                    for dx in range(3):
                        if (dy, dx) == (1, 1):
                            continue
                        oy0, oy1 = max(0, 1-dy), min(16, 17-dy)
                        ox0, ox1 = max(0, 1-dx), min(16, 17-dx)
                        src = x1_3[:, oy0+dy-1:oy1+dy-1, ox0+dx-1:ox1+dx-1]
                        dsts = a3[:, oy0:oy1, ox0:ox1]
                        ct = (ch * 9 + 3 * dy + dx) * 4 + t
                        VE.scalar_tensor_tensor(
                            out=dsts, in0=src, scalar=dwt[:, ct:ct+1],
                            in1=dsts, op0=AL.mult, op1=AL.add)
                U = lif_add(m_dw[ch], acc, t)
                mg = mg_p.tile([P, N], F32, name=f"mg{t}_{ch}", tag="t")
                VE.scalar_tensor_tensor(out=mg, in0=U, scalar=thr, in1=x2s,
                                        op0=AL.is_ge, op1=AL.mult)
                lif_reset(m_dw[ch], U, t)
                for mt in range(3):
                    nc.tensor.matmul(po[mt], f2_w[ch][:, mt*P:(mt+1)*P], mg,
                                     start=(ch == 0), stop=(ch == 7),
                                     skip_group_check=True)
            # fc2 evict + final LIF + residual + store
            for mt in range(3):
                xo = xo_p.tile([P, N], F32, name=f"xo{t}_{mt}", tag="t")
                c = mt * 4 + t
                nc.scalar.activation(xo, po[mt], AF.Identity,
                                     bias=b_o[:, c:c+1], scale=a_o[:, c:c+1])
                U = lif_add(m_o[mt], xo, t)
                ob = osb_p.tile([P, N], F32, name=f"ob{t}_{mt}", tag="t")
                VE.scalar_tensor_tensor(out=ob, in0=U, scalar=thr, in1=x_new[mt],
                                        op0=AL.is_ge, op1=AL.add)
                lif_reset(m_o[mt], U, t)
                nc.sync.dma_start(out=d['out'][t*C + mt*P: t*C + (mt+1)*P, :], in_=ob)


def _build():
    nc = bacc.Bacc()
    with tile.TileContext(nc) as tc:
        with tc.tile_pool(name="dram", bufs=1, space="DRAM") as dram:
            def din(name, shape, dt=F32):
                return dram.tile(shape, dt, kind="ExternalInput", name=name,
                                 uniquify=False)
            d = {
                'xin': din('xin', [T * C, N]),
                'out': dram.tile([T * C, N], F32, kind="ExternalOutput",
                                 name='out', uniquify=False),
                'kq_wT': din('kq_wT', [384, 480]),
                'a_kq': din('a_kq', [96, 20]),
                'b_kq': din('b_kq', [96, 20]),
                'v_wT': din('v_wT', [384, 384]),
                'r_wT': din('r_wT', [384, 4]),
                'r_b': din('r_b', [1, 4]),
                'ones': din('ones', [1, 128]),
                'proj_wT': din('proj_wT', [384, 384]),
                'a_p': din('a_p', [128, 12]),
                'b_p': din('b_p', [128, 12]),
                'fc1_wT': din('fc1_wT', [384, 2048]),
                'a_h': din('a_h', [128, 64]),
                'b_h': din('b_h', [128, 64]),
                'dw_tap': din('dw_tap', [128, 288]),
                'b_dw': din('b_dw', [128, 32]),
                'fc2_wT': din('fc2_wT', [1024, 384]),
                'a_o': din('a_o', [128, 12]),
                'b_o': din('b_o', [128, 12]),
                'ident': din('ident', [128, 128], BF16),
            }
            _body(nc, tc, d)
    nc.finalize()
    return nc


_NC_CACHE = {}


def _get_nc():
    if 'nc' not in _NC_CACHE:
        _NC_CACHE['nc'] = _build()
    return _NC_CACHE['nc']


def _tcols(a):
    """(rows, k) per-unit scalars -> (rows, 4k) with column u*4+t = 2^t * a[:,u]."""
    rows, k = a.shape
    out = np.empty((rows, k * 4), np.float32)
    for u in range(k):
        for t in range(4):
            out[:, u * 4 + t] = a[:, u] * (2.0 ** t)
    return out


def _prep_common(inputs):
    inp = {k: np.asarray(v, np.float32) for k, v in inputs.items()}
    k_wT = inp['k_w'].T
    exp_wT = np.concatenate([inp['exp_w'][e].T for e in range(NE)], axis=1)
    kq_wT = np.concatenate([k_wT, exp_wT], axis=1)
    a_kq = np.zeros((96, 5), np.float32)
    b_kq = np.zeros((96, 5), np.float32)
    a_kq[:, 0] = 0.5
    for e in range(NE):
        a_kq[:, 1 + e] = 0.5 * inp['exp_g'][e] * S
        b_kq[:, 1 + e] = 0.5 * inp['exp_b'][e]
    taps = inp['dw_w'][:, 0] * (0.5 * inp['dw_g'] * S)[:, None, None]
    com = {
        'kq_wT': kq_wT,
        'a_kq': _tcols(a_kq), 'b_kq': _tcols(b_kq),
        'v_wT': inp['v_w'].T,
        'r_wT': inp['router_w'].T * (inp['router_g'] * S * 0.5)[None, :],
        'r_b': (0.5 * (inp['router_b'] * inp['router_g'] * S
                       + inp['router_be'])).reshape(1, 4),
        'ones': np.ones((1, 128), np.float32),
        'proj_wT': inp['proj_w'].T,
        'a_p': _tcols((0.5 * inp['proj_g'] * S).reshape(3, 128).T),
        'b_p': _tcols((0.5 * (inp['proj_b'] * inp['proj_g'] * S
                              + inp['proj_be'])).reshape(3, 128).T),
        'fc1_wT': inp['fc1_w'].T,
        'a_h': _tcols((0.5 * inp['fc1_g'] * S).reshape(16, 128).T),
        'b_h': _tcols((0.5 * (inp['fc1_b'] * inp['fc1_g'] * S
                              + inp['fc1_be'])).reshape(16, 128).T),
        'dw_tap': _tcols(taps.reshape(8, 128, 9).transpose(1, 0, 2).reshape(128, 72)),
        'b_dw': _tcols((0.5 * (inp['dw_b'] * inp['dw_g'] * S
                               + inp['dw_be'])).reshape(8, 128).T),
        'fc2_wT': inp['fc2_w'].T,
        'a_o': _tcols((0.5 * inp['fc2_g'] * S).reshape(3, 128).T),
        'b_o': _tcols((0.5 * (inp['fc2_b'] * inp['fc2_g'] * S
                              + inp['fc2_be'])).reshape(3, 128).T),
        'ident': np.eye(128, dtype=ml_dtypes.bfloat16),
    }
    return {k: np.ascontiguousarray(v) for k, v in com.items()}


def run(inputs, trace=False, tmpdir=None):
    com = _prep_common(inputs)
    x = np.asarray(inputs['x'], np.float32).reshape(T, B, C, N)
    in_maps = []
    for b in range(B):
        m = dict(com)
        m['xin'] = np.ascontiguousarray(x[:, b].reshape(T * C, N))
        in_maps.append(m)
    res = run_bass_kernel_spmd(_get_nc(), in_maps, list(range(B)),
                               trace=trace, tmpdir=tmpdir)
    out = np.empty((T, B, C, N), np.float32)
    for b in range(B):
        out[:, b] = res.results[b]['out'].reshape(T, C, N)
    return out.reshape(T * B, C, 16, 16), res.exec_time_ns


def kernel(**inputs):
    out, _ = run(inputs)
    return out


# revision 9
# speedup vs baseline: 4587.8614x; 1.0470x over previous
"""Trainium2 Bass kernel for nn_Block_31954556682442 (spiking MoE-SSA block).

Sharding: pure data-parallel over batch B=8 -> one sample (4 LIF time steps)
per NeuronCore, zero collectives. v2 design:
  - all weight matmuls as bf16 hi/lo split (3-term W@x for fc1/kq/v with
    bf16-split activations; 2-term for proj/fc2 whose rhs are exact bf16
    integers), residual error ~2^-18 -> no spike flips observed
  - time steps batched into matmul free dims (N=512 covers 2 steps)
  - bf16 exact-integer attention core (spikes are {0,1})
  - LIF scans in 2^t-scaled form: membrane update = tensor_add on GPSIMD,
    spike/reset = tensor_scalar/scalar_tensor_tensor on DVE (threshold 2^t)
  - depthwise 3x3 conv t-batched: 9 shifted per-partition-scalar MACs over
    (128, 4*256) tiles on DVE, 2^t applied at the LIF add
  - PSUM evicts fused with BN scale+bias (+2^t*0.5) on ScalarE
Self-contained: hardcodes all shapes; no sibling imports.
"""
import numpy as np
import ml_dtypes

import concourse.bacc as bacc
import concourse.mybir as mybir
import concourse.tile as tile
from concourse.bass_utils import run_bass_kernel_spmd

F32 = mybir.dt.float32
BF16 = mybir.dt.bfloat16
AL = mybir.AluOpType
AF = mybir.ActivationFunctionType

T, B, C, N = 4, 8, 384, 256
ED = 96
NE = 4
NU = 5
HID, HH = 2048, 1024
S = float(1.0 / np.sqrt(1.0 + 1e-5))
P = 128


def _body(nc, tc, d):
    from contextlib import ExitStack
    VE = nc.vector
    GE = nc.gpsimd

    with ExitStack() as ctx:
        def pool(name, bufs, space="SBUF"):
            return ctx.enter_context(tc.tile_pool(name=name, bufs=bufs, space=space))

        wp = pool("wp", 1)
        mp = pool("mp", 1)
        ps_m = pool("ps_m", 2, "PSUM")
        ps_o = pool("ps_o", 6, "PSUM")
        xs_p = pool("xs_p", 3)       # (128,1024) f32, doubles as x_new
        sphl_p = pool("sphl_p", 3)   # bf16 hi splits
        splo_p = pool("splo_p", 3)   # bf16 lo splits
        xkq_p = pool("xkq_p", 2)     # (96,1280) f32
        xev_p = pool("xev_p", 4)     # (128,768) f32 evict/LIF targets
        xrt_p = pool("xrt_p", 2)     # (128,8)
        sp_p = pool("sp_p", 4)       # (96,1280) bf16 kq spikes
        vsp_p = pool("vsp_p", 4)     # (128,768) bf16
        wsp_p = pool("wsp_p", 4)     # (128,8) f32
        at_p = pool("at_p", 3)       # (128,256) bf16
        rsp_p = pool("rsp_p", 2)     # (128,768) bf16
        y_p = pool("y_p", 8)         # (128,384) bf16
        ydn_p = pool("ydn_p", 3)     # (128,1024) bf16
        xh_p = pool("xh_p", 2)       # (128,2048) f32
        spch_p = pool("spch_p", 2)   # (128,2048) bf16
        acc_p = pool("acc_p", 2)     # (128,1024) f32
        mg_p = pool("mg_p", 2)       # (128,1024) bf16
        mh_p = pool("mh_p", 2)       # (128,512) f32
        mdw_p = pool("mdw_p", 2)     # (128,256) f32

        # ---------------- weight loads ----------------
        def wload(name, shape, dt=F32, src=None):
            w = wp.tile(shape, dt, name=name, tag=name)
            nc.sync.dma_start(out=w, in_=d[name] if src is None else src)
            return w

        # xs first (A-phase starts on these)
        xs_kt = []
        for kt in range(3):
            x_ = xs_p.tile([P, 4 * N], F32, name=f"xs{kt}", tag="t")
            xs_kt.append(x_)
        for t in range(T):
            for kt in range(3):
                nc.sync.dma_start(out=xs_kt[kt][:, t*N:(t+1)*N],
                                  in_=d['xin'][t*C + kt*P: t*C + (kt+1)*P, :])
        kqh, kql, vh, vl, r_w = [], [], [], [], []
        for kt in range(3):
            kqh.append(wload(f'kqh{kt}', [P, 480], BF16, d['kq_whi'][kt*P:(kt+1)*P, :]))
            kql.append(wload(f'kql{kt}', [P, 480], BF16, d['kq_wlo'][kt*P:(kt+1)*P, :]))
        a_kq = wload('a_kq', [96, 20]); b_kq = wload('b_kq', [96, 20])
        for kt in range(3):
            vh.append(wload(f'vh{kt}', [P, 384], BF16, d['v_whi'][kt*P:(kt+1)*P, :]))
            vl.append(wload(f'vl{kt}', [P, 384], BF16, d['v_wlo'][kt*P:(kt+1)*P, :]))
            r_w.append(wload(f'r_w{kt}', [P, 4], F32, d['r_wT'][kt*P:(kt+1)*P, :]))
        rb = wload('r_b', [1, 4]); ones = wload('ones', [1, P])
        ident = wload('ident', [P, P], BF16)
        pjh, pjl, f1h, f1l, f2h, f2l = [], [], [], [], [], []
        for kt in range(3):
            pjh.append(wload(f'pjh{kt}', [P, 384], BF16, d['pj_whi'][kt*P:(kt+1)*P, :]))
            pjl.append(wload(f'pjl{kt}', [P, 384], BF16, d['pj_wlo'][kt*P:(kt+1)*P, :]))
        a_p = wload('a_p', [P, 12]); b_p = wload('b_p', [P, 12])
        for kt in range(3):
            f1h.append(wload(f'f1h{kt}', [P, 2048], BF16, d['f1_whi'][kt*P:(kt+1)*P, :]))
            f1l.append(wload(f'f1l{kt}', [P, 2048], BF16, d['f1_wlo'][kt*P:(kt+1)*P, :]))
        a_h = wload('a_h', [P, 64]); b_h = wload('b_h', [P, 64])
        dwt = wload('dw_tap', [P, 72]); b_dw = wload('b_dw', [P, 8])
        for ch in range(8):
            f2h.append(wload(f'f2h{ch}', [P, 384], BF16, d['f2_whi'][ch*P:(ch+1)*P, :]))
            f2l.append(wload(f'f2l{ch}', [P, 384], BF16, d['f2_wlo'][ch*P:(ch+1)*P, :]))
        a_o = wload('a_o', [P, 12]); b_o = wload('b_o', [P, 12])

        # ---------------- xs bf16 splits ----------------
        xhi, xlo = [], []
        for kt in range(3):
            h_ = sphl_p.tile([P, 4 * N], BF16, name=f"xhi{kt}", tag="t")
            nc.scalar.activation(h_, xs_kt[kt], AF.Copy)
            l_ = splo_p.tile([P, 4 * N], BF16, name=f"xlo{kt}", tag="t")
            VE.tensor_sub(l_, xs_kt[kt], h_)
            xhi.append(h_); xlo.append(l_)

        # ---------------- phase A: kq / v / router matmuls + evicts ----------------
        m_kq = mp.tile([96, 5 * N], F32, name="m_kq", tag="m_kq")
        m_vt = mp.tile([P, 768], F32, name="m_vt", tag="m_vt")
        m_rt = mp.tile([P, 8], F32, name="m_rt", tag="m_rt")
        m_res = mp.tile([P, 768], F32, name="m_res", tag="m_res")
        m_p = mp.tile([P, 768], F32, name="m_p", tag="m_p")
        m_o = mp.tile([P, 768], F32, name="m_o", tag="m_o")

        xkq_t = [xkq_p.tile([96, 5 * N], F32, name=f"xkq{t}", tag="t") for t in range(T)]
        xvt_t = [xev_p.tile([P, 768], F32, name=f"xvt{t}", tag="t") for t in range(T)]
        xrt_t = [xrt_p.tile([P, 8], F32, name=f"xrt{t}", tag="t") for t in range(T)]

        for tp in range(2):
            for u in range(NU):
                pt = ps_m.tile([96, 512], F32, name=f"pkq{u}_{tp}", tag="pm")
                first = True
                for kt in range(3):
                    rh = xhi[kt][:, tp*512:(tp+1)*512]
                    rl = xlo[kt][:, tp*512:(tp+1)*512]
                    for w_, r_ in ((kqh[kt], rh), (kqh[kt], rl), (kql[kt], rh)):
                        nc.tensor.matmul(pt, w_[:, 96*u:96*(u+1)], r_,
                                         start=first, stop=(kt == 2 and r_ is rh and w_ is kql[kt]))
                        first = False
                for ti in range(2):
                    t = tp * 2 + ti
                    c = u * 4 + t
                    nc.scalar.activation(xkq_t[t][:, u*N:(u+1)*N], pt[:, ti*N:(ti+1)*N],
                                         AF.Identity, bias=b_kq[:, c:c+1], scale=a_kq[:, c:c+1])
        for t in range(T):
            for mt in range(2):
                pv = ps_m.tile([P, 384], F32, name=f"pvt{t}_{mt}", tag="pm")
                first = True
                for kt in range(3):
                    lh = xhi[kt][:, t*N + mt*P: t*N + (mt+1)*P]
                    ll = xlo[kt][:, t*N + mt*P: t*N + (mt+1)*P]
                    for l_, w_ in ((lh, vh[kt]), (ll, vh[kt]), (lh, vl[kt])):
                        nc.tensor.matmul(pv, l_, w_, start=first,
                                         stop=(kt == 2 and l_ is lh and w_ is vl[kt]))
                        first = False
                nc.scalar.activation(xvt_t[t][:, mt*384:(mt+1)*384], pv, AF.Copy,
                                     bias=0.0, scale=0.5 * float(2.0 ** t))
            for mt in range(2):
                pr = ps_m.tile([P, 4], F32, name=f"prt{t}_{mt}", tag="pm")
                for kt in range(3):
                    nc.tensor.matmul(pr, xs_kt[kt][:, t*N + mt*P: t*N + (mt+1)*P],
                                     r_w[kt], start=(kt == 0), stop=False)
                nc.tensor.matmul(pr, ones, rb, start=False, stop=True)
                nc.scalar.activation(xrt_t[t][:, mt*4:(mt+1)*4], pr, AF.Copy,
                                     bias=0.0, scale=float(2.0 ** t))

        # ---------------- phase B: LIF scans for kq / v / r ----------------
        sp_t, v_sp, w_sp = [], [], []
        for t in range(T):
            thr = float(2.0 ** t)
            U = xkq_t[t]
            if t > 0:
                GE.tensor_add(U, m_kq, U)
            sp = sp_p.tile([96, 5 * N], BF16, name=f"sp{t}", tag="t")
            VE.tensor_single_scalar(sp, U, thr, AL.is_ge)
            if t < T - 1:
                VE.scalar_tensor_tensor(out=m_kq, in0=U, scalar=thr, in1=U,
                                        op0=AL.is_lt, op1=AL.mult)
            sp_t.append(sp)

            U = xvt_t[t]
            if t > 0:
                GE.tensor_add(U, m_vt, U)
            vs = vsp_p.tile([P, 768], BF16, name=f"vsp{t}", tag="t")
            VE.tensor_single_scalar(vs, U, thr, AL.is_ge)
            if t < T - 1:
                VE.scalar_tensor_tensor(out=m_vt, in0=U, scalar=thr, in1=U,
                                        op0=AL.is_lt, op1=AL.mult)
            v_sp.append(vs)

            U = xrt_t[t]
            if t > 0:
                GE.tensor_add(U, m_rt, U)
            ws = wsp_p.tile([P, 8], F32, name=f"wsp{t}", tag="t")
            VE.tensor_single_scalar(ws, U, thr, AL.is_ge)
            if t < T - 1:
                VE.scalar_tensor_tensor(out=m_rt, in0=U, scalar=thr, in1=U,
                                        op0=AL.is_lt, op1=AL.mult)
            w_sp.append(ws)

        # ---------------- phase C: experts ----------------
        y = [[None] * 2 for _ in range(T)]
        for e in range(NE):
            xres_e = []
            for t in range(T):
                at_sb = []
                for mt in range(2):
                    pa = ps_m.tile([P, N], F32, name=f"pat{e}{t}{mt}", tag="pm")
                    nc.tensor.matmul(pa, sp_t[t][:, mt*P:(mt+1)*P],
                                     sp_t[t][:, (1+e)*N:(2+e)*N], start=True, stop=True)
                    ats = at_p.tile([P, N], BF16, name=f"at{e}{t}{mt}", tag="t")
                    nc.scalar.activation(ats, pa, AF.Copy)
                    at_sb.append(ats)
                xr = xev_p.tile([P, 768], F32, name=f"xres{e}{t}", tag="t")
                for mt in range(2):
                    pr_ = ps_m.tile([P, 384], F32, name=f"pres{e}{t}{mt}", tag="pm")
                    for mk in range(2):
                        nc.tensor.matmul(pr_, at_sb[mk][:, mt*P:(mt+1)*P],
                                         v_sp[t][:, mk*384:(mk+1)*384],
                                         start=(mk == 0), stop=(mk == 1))
                    nc.scalar.activation(xr[:, mt*384:(mt+1)*384], pr_, AF.Copy,
                                         bias=0.0, scale=0.5 * float(2.0 ** t))
                xres_e.append(xr)
            for t in range(T):
                thr = float(2.0 ** t)
                U = xres_e[t]
                if t > 0:
                    GE.tensor_add(U, m_res, U)
                rs = rsp_p.tile([P, 768], BF16, name=f"rsp{e}{t}", tag="t")
                VE.tensor_single_scalar(rs, U, thr, AL.is_ge)
                if t < T - 1:
                    VE.scalar_tensor_tensor(out=m_res, in0=U, scalar=thr, in1=U,
                                            op0=AL.is_lt, op1=AL.mult)
                for mt in range(2):
                    if e == 0:
                        yt = y_p.tile([P, 384], BF16, name=f"y{t}_{mt}", tag="t")
                        VE.scalar_tensor_tensor(
                            out=yt, in0=rs[:, mt*384:(mt+1)*384],
                            scalar=w_sp[t][:, mt*4:mt*4+1],
                            in1=rs[:, mt*384:(mt+1)*384], op0=AL.mult, op1=AL.bypass)
                        y[t][mt] = yt
                    else:
                        VE.scalar_tensor_tensor(
                            out=y[t][mt], in0=rs[:, mt*384:(mt+1)*384],
                            scalar=w_sp[t][:, mt*4+e:mt*4+e+1],
                            in1=y[t][mt], op0=AL.mult, op1=AL.add)

        # ---------------- phase D: transpose y, proj, LIF, residual ----------------
        ydn = [ydn_p.tile([P, 4 * N], BF16, name=f"ydn{dt}", tag="t") for dt in range(3)]
        for t in range(T):
            for mt in range(2):
                for dt in range(3):
                    ptr = ps_m.tile([P, P], BF16, name=f"ptr{t}{mt}{dt}", tag="pm")
                    nc.tensor.transpose(ptr, y[t][mt][:, dt*P:(dt+1)*P], ident)
                    nc.scalar.activation(ydn[dt][:, t*N + mt*P: t*N + (mt+1)*P],
                                         ptr, AF.Copy)
        xp_t = [xev_p.tile([P, 768], F32, name=f"xp{t}", tag="t") for t in range(T)]
        for mt in range(3):
            for tp in range(2):
                pp = ps_m.tile([P, 512], F32, name=f"pp{mt}_{tp}", tag="pm")
                first = True
                for kt in range(3):
                    r_ = ydn[kt][:, tp*512:(tp+1)*512]
                    nc.tensor.matmul(pp, pjh[kt][:, mt*P:(mt+1)*P], r_,
                                     start=first, stop=False)
                    first = False
                    nc.tensor.matmul(pp, pjl[kt][:, mt*P:(mt+1)*P], r_,
                                     start=False, stop=(kt == 2))
                for ti in range(2):
                    t = tp * 2 + ti
                    c = mt * 4 + t
                    nc.scalar.activation(xp_t[t][:, mt*N:(mt+1)*N], pp[:, ti*N:(ti+1)*N],
                                         AF.Identity, bias=b_p[:, c:c+1], scale=a_p[:, c:c+1])
        for t in range(T):
            thr = float(2.0 ** t)
            U = xp_t[t]
            if t > 0:
                GE.tensor_add(U, m_p, U)
            if t < T - 1:
                VE.scalar_tensor_tensor(out=m_p, in0=U, scalar=thr, in1=U,
                                        op0=AL.is_lt, op1=AL.mult)
            for mt in range(3):
                # x_new overwrites xs in place (residual add)
                VE.scalar_tensor_tensor(
                    out=xs_kt[mt][:, t*N:(t+1)*N], in0=U[:, mt*N:(mt+1)*N],
                    scalar=thr, in1=xs_kt[mt][:, t*N:(t+1)*N],
                    op0=AL.is_ge, op1=AL.add)

        # x_new bf16 splits (reuses the split pool slots)
        xnhi, xnlo = [], []
        for kt in range(3):
            h_ = sphl_p.tile([P, 4 * N], BF16, name=f"xnhi{kt}", tag="t")
            nc.scalar.activation(h_, xs_kt[kt], AF.Copy)
            l_ = splo_p.tile([P, 4 * N], BF16, name=f"xnlo{kt}", tag="t")
            VE.tensor_sub(l_, xs_kt[kt], h_)
            xnhi.append(h_); xnlo.append(l_)

        # ---------------- phase E: MLP ----------------
        po = [[ps_o.tile([P, 512], F32, name=f"po{tp}_{mt}", tag="po")
               for mt in range(3)] for tp in range(2)]
        for ch in range(8):
            xh = xh_p.tile([P, 2048], F32, name=f"xh{ch}", tag="t")
            for half in range(2):
                mth = ch + 8 * half
                for tp in range(2):
                    ph = ps_m.tile([P, 512], F32, name=f"ph{ch}{half}{tp}", tag="pm")
                    first = True
                    for kt in range(3):
                        rh = xnhi[kt][:, tp*512:(tp+1)*512]
                        rl = xnlo[kt][:, tp*512:(tp+1)*512]
                        for w_, r_ in ((f1h[kt], rh), (f1h[kt], rl), (f1l[kt], rh)):
                            nc.tensor.matmul(ph, w_[:, mth*P:(mth+1)*P], r_,
                                             start=first,
                                             stop=(kt == 2 and r_ is rh and w_ is f1l[kt]))
                            first = False
                    for ti in range(2):
                        t = tp * 2 + ti
                        c = mth * 4 + t
                        nc.scalar.activation(
                            xh[:, half*1024 + t*N: half*1024 + (t+1)*N],
                            ph[:, ti*N:(ti+1)*N], AF.Identity,
                            bias=b_h[:, c:c+1], scale=a_h[:, c:c+1])
            # h-LIF over t (both halves via 3D APs)
            m_h = mh_p.tile([P, 512], F32, name=f"m_h{ch}", tag="t")
            sp_ch = spch_p.tile([P, 2048], BF16, name=f"spch{ch}", tag="t")
            xh3 = xh.rearrange("p (h q) -> p h q", h=2)
            mh3 = m_h.rearrange("p (h q) -> p h q", h=2)
            spc3 = sp_ch.rearrange("p (h q) -> p h q", h=2)
            for t in range(T):
                thr = float(2.0 ** t)
                U3 = xh3[:, :, t*N:(t+1)*N]
                if t > 0:
                    GE.tensor_add(U3, mh3, U3)
                VE.tensor_single_scalar(spc3[:, :, t*N:(t+1)*N], U3, thr, AL.is_ge)
                if t < T - 1:
                    VE.scalar_tensor_tensor(out=mh3, in0=U3, scalar=thr, in1=U3,
                                            op0=AL.is_lt, op1=AL.mult)
            # depthwise conv, t-batched, unscaled taps
            acc = acc_p.tile([P, 1024], F32, name=f"acc{ch}", tag="t")
            VE.tensor_scalar(acc, sp_ch[:, 0:1024], dwt[:, ch*9+4:ch*9+5],
                             b_dw[:, ch:ch+1], AL.mult, AL.add)
            x1f = sp_ch[:, 0:1024]
            x1r = x1f.rearrange("p (r w) -> p r w", w=16)     # 64 rows across t
            x1t = x1f.rearrange("p (t r) -> p t r", t=4)      # 4 t-blocks of 256
            ar = acc.rearrange("p (r w) -> p r w", w=16)
            at4 = acc.rearrange("p (t r) -> p t r", t=4)
            for dy in range(3):
                for dx in range(3):
                    if (dy, dx) == (1, 1):
                        continue
                    ct = ch * 9 + 3 * dy + dx
                    sc = dwt[:, ct:ct+1]
                    if dy == 1:
                        # pure x-shift: rows uniform across all t
                        wo0, wo1 = (1, 16) if dx == 0 else (0, 15)
                        VE.scalar_tensor_tensor(
                            out=ar[:, :, wo0:wo1], in0=x1r[:, :, wo0+dx-1:wo1+dx-1],
                            scalar=sc, in1=ar[:, :, wo0:wo1], op0=AL.mult, op1=AL.add)
                    elif dx == 1:
                        # pure y-shift: contiguous 240-element run per t-block
                        ho0 = 1 if dy == 0 else 0
                        o0 = ho0 * 16
                        i0 = o0 + (dy - 1) * 16
                        VE.scalar_tensor_tensor(
                            out=at4[:, :, o0:o0+240], in0=x1t[:, :, i0:i0+240],
                            scalar=sc, in1=at4[:, :, o0:o0+240], op0=AL.mult, op1=AL.add)
                    else:
                        # corner: per-t (p, 15, 15)
                        ho0 = 1 if dy == 0 else 0
                        wo0 = 1 if dx == 0 else 0
                        for t4 in range(T):
                            o3 = acc[:, t4*N:(t4+1)*N].rearrange("p (h w) -> p h w", w=16)
                            i3 = x1f[:, t4*N:(t4+1)*N].rearrange("p (h w) -> p h w", w=16)
                            VE.scalar_tensor_tensor(
                                out=o3[:, ho0:ho0+15, wo0:wo0+15],
                                in0=i3[:, ho0+dy-1:ho0+dy+14, wo0+dx-1:wo0+dx+14],
                                scalar=sc, in1=o3[:, ho0:ho0+15, wo0:wo0+15],
                                op0=AL.mult, op1=AL.add)
            # dw-LIF + gate -> mg (bf16)
            m_dw = mdw_p.tile([P, N], F32, name=f"m_dw{ch}", tag="t")
            mg = mg_p.tile([P, 1024], BF16, name=f"mg{ch}", tag="t")
            for t in range(T):
                thr = float(2.0 ** t)
                U = acc[:, t*N:(t+1)*N]
                if t > 0:
                    VE.scalar_tensor_tensor(out=U, in0=U, scalar=thr, in1=m_dw,
                                            op0=AL.mult, op1=AL.add)
                VE.scalar_tensor_tensor(out=mg[:, t*N:(t+1)*N], in0=U, scalar=thr,
                                        in1=sp_ch[:, 1024 + t*N: 1024 + (t+1)*N],
                                        op0=AL.is_ge, op1=AL.mult)
                if t < T - 1:
                    VE.scalar_tensor_tensor(out=m_dw, in0=U, scalar=thr, in1=U,
                                            op0=AL.is_lt, op1=AL.mult)
            # fc2 accumulate (2-term bf16, rhs exact)
            for tp in range(2):
                for mt in range(3):
                    nc.tensor.matmul(po[tp][mt], f2h[ch][:, mt*P:(mt+1)*P],
                                     mg[:, tp*512:(tp+1)*512],
                                     start=(ch == 0), stop=False, skip_group_check=True)
                    nc.tensor.matmul(po[tp][mt], f2l[ch][:, mt*P:(mt+1)*P],
                                     mg[:, tp*512:(tp+1)*512],
                                     start=False, stop=(ch == 7), skip_group_check=True)

        # fc2 evict + final LIF + residual + store
        xo_t = [xev_p.tile([P, 768], F32, name=f"xo{t}", tag="t") for t in range(T)]
        for t in range(T):
            for mt in range(3):
                c = mt * 4 + t
                nc.scalar.activation(xo_t[t][:, mt*N:(mt+1)*N],
                                     po[t // 2][mt][:, (t % 2)*N:(t % 2+1)*N],
                                     AF.Identity, bias=b_o[:, c:c+1], scale=a_o[:, c:c+1])
        for t in range(T):
            thr = float(2.0 ** t)
            U = xo_t[t]
            if t > 0:
                GE.tensor_add(U, m_o, U)
            if t < T - 1:
                VE.scalar_tensor_tensor(out=m_o, in0=U, scalar=thr, in1=U,
                                        op0=AL.is_lt, op1=AL.mult)
            for mt in range(3):
                # final out in place over xo (reset already consumed U)
                VE.scalar_tensor_tensor(
                    out=U[:, mt*N:(mt+1)*N], in0=U[:, mt*N:(mt+1)*N], scalar=thr,
                    in1=xs_kt[mt][:, t*N:(t+1)*N], op0=AL.is_ge, op1=AL.add)
                nc.sync.dma_start(out=d['out'][t*C + mt*P: t*C + (mt+1)*P, :],
                                  in_=U[:, mt*N:(mt+1)*N])


def _build():
    nc = bacc.Bacc()
    with tile.TileContext(nc) as tc:
        with tc.tile_pool(name="dram", bufs=1, space="DRAM") as dram:
            def din(name, shape, dt=F32):
                return dram.tile(shape, dt, kind="ExternalInput", name=name,
                                 uniquify=False)
            d = {
                'xin': din('xin', [T * C, N]),
                'out': dram.tile([T * C, N], F32, kind="ExternalOutput",
                                 name='out', uniquify=False),
                'kq_whi': din('kq_whi', [384, 480], BF16),
                'kq_wlo': din('kq_wlo', [384, 480], BF16),
                'a_kq': din('a_kq', [96, 20]),
                'b_kq': din('b_kq', [96, 20]),
                'v_whi': din('v_whi', [384, 384], BF16),
                'v_wlo': din('v_wlo', [384, 384], BF16),
                'r_wT': din('r_wT', [384, 4]),
                'r_b': din('r_b', [1, 4]),
                'ones': din('ones', [1, 128]),
                'pj_whi': din('pj_whi', [384, 384], BF16),
                'pj_wlo': din('pj_wlo', [384, 384], BF16),
                'a_p': din('a_p', [128, 12]),
                'b_p': din('b_p', [128, 12]),
                'f1_whi': din('f1_whi', [384, 2048], BF16),
                'f1_wlo': din('f1_wlo', [384, 2048], BF16),
                'a_h': din('a_h', [128, 64]),
                'b_h': din('b_h', [128, 64]),
                'dw_tap': din('dw_tap', [128, 72]),
                'b_dw': din('b_dw', [128, 8]),
                'f2_whi': din('f2_whi', [1024, 384], BF16),
                'f2_wlo': din('f2_wlo', [1024, 384], BF16),
                'a_o': din('a_o', [128, 12]),
                'b_o': din('b_o', [128, 12]),
                'ident': din('ident', [128, 128], BF16),
            }
            _body(nc, tc, d)
    nc.finalize()
    return nc


_NC_CACHE = {}


def _get_nc():
    if 'nc' not in _NC_CACHE:
        _NC_CACHE['nc'] = _build()
    return _NC_CACHE['nc']


def _tcols(a):
    rows, k = a.shape
    out = np.empty((rows, k * 4), np.float32)
    for u in range(k):
        for t in range(4):
            out[:, u * 4 + t] = a[:, u] * (2.0 ** t)
    return out


def _split(w):
    hi = w.astype(ml_dtypes.bfloat16)
    lo = (w - hi.astype(np.float32)).astype(ml_dtypes.bfloat16)
    return hi, lo


def _prep_common(inputs):
    inp = {k: np.asarray(v, np.float32) for k, v in inputs.items()}
    k_wT = inp['k_w'].T
    exp_wT = np.concatenate([inp['exp_w'][e].T for e in range(NE)], axis=1)
    kq_wT = np.concatenate([k_wT, exp_wT], axis=1)
    a_kq = np.zeros((96, 5), np.float32)
    b_kq = np.zeros((96, 5), np.float32)
    a_kq[:, 0] = 0.5
    for e in range(NE):
        a_kq[:, 1 + e] = 0.5 * inp['exp_g'][e] * S
        b_kq[:, 1 + e] = 0.5 * inp['exp_b'][e]
    taps = inp['dw_w'][:, 0] * (0.5 * inp['dw_g'] * S)[:, None, None]
    kqh, kqlo = _split(kq_wT)
    vhh, vlo = _split(inp['v_w'].T)
    pjh_, pjlo = _split(inp['proj_w'].T)
    f1h_, f1lo = _split(inp['fc1_w'].T)
    f2h_, f2lo = _split(inp['fc2_w'].T)
    com = {
        'kq_whi': kqh, 'kq_wlo': kqlo,
        'a_kq': _tcols(a_kq), 'b_kq': _tcols(b_kq),
        'v_whi': vhh, 'v_wlo': vlo,
        'r_wT': inp['router_w'].T * (inp['router_g'] * S * 0.5)[None, :],
        'r_b': (0.5 * (inp['router_b'] * inp['router_g'] * S
                       + inp['router_be'])).reshape(1, 4),
        'ones': np.ones((1, 128), np.float32),
        'pj_whi': pjh_, 'pj_wlo': pjlo,
        'a_p': _tcols((0.5 * inp['proj_g'] * S).reshape(3, 128).T),
        'b_p': _tcols((0.5 * (inp['proj_b'] * inp['proj_g'] * S
                              + inp['proj_be'])).reshape(3, 128).T),
        'f1_whi': f1h_, 'f1_wlo': f1lo,
        'a_h': _tcols((0.5 * inp['fc1_g'] * S).reshape(16, 128).T),
        'b_h': _tcols((0.5 * (inp['fc1_b'] * inp['fc1_g'] * S
                              + inp['fc1_be'])).reshape(16, 128).T),
        'dw_tap': taps.reshape(8, 128, 9).transpose(1, 0, 2).reshape(128, 72),
        'b_dw': (0.5 * (inp['dw_b'] * inp['dw_g'] * S
                        + inp['dw_be'])).reshape(8, 128).T,
        'f2_whi': f2h_, 'f2_wlo': f2lo,
        'a_o': _tcols((0.5 * inp['fc2_g'] * S).reshape(3, 128).T),
        'b_o': _tcols((0.5 * (inp['fc2_b'] * inp['fc2_g'] * S
                              + inp['fc2_be'])).reshape(3, 128).T),
        'ident': np.eye(128, dtype=ml_dtypes.bfloat16),
    }
    return {k: np.ascontiguousarray(v) for k, v in com.items()}


def run(inputs, trace=False, tmpdir=None):
    com = _prep_common(inputs)
    x = np.asarray(inputs['x'], np.float32).reshape(T, B, C, N)
    in_maps = []
    for b in range(B):
        m = dict(com)
        m['xin'] = np.ascontiguousarray(x[:, b].reshape(T * C, N))
        in_maps.append(m)
    res = run_bass_kernel_spmd(_get_nc(), in_maps, list(range(B)),
                               trace=trace, tmpdir=tmpdir)
    out = np.empty((T, B, C, N), np.float32)
    for b in range(B):
        out[:, b] = res.results[b]['out'].reshape(T, C, N)
    return out.reshape(T * B, C, 16, 16), res.exec_time_ns


def kernel(**inputs):
    out, _ = run(inputs)
    return out
